# revision 1
# baseline (speedup 1.0000x reference)
"""GCN dialogue manager kernel for 8 trn2 NeuronCores.

Math (reference):
    h   = x @ W_gcn
    deg = in_deg(dst) + 1   (self loops)
    dinv = rsqrt(deg)
    agg[d] = sum_{e:(s->d)} dinv[s]*dinv[d]*h[s] + dinv[d]^2*h[d] + b_gcn
    out = agg @ W_act + b_act

Kernel strategy (dst-sharded, edges partitioned by destination):
    out[d] = dinv[d] * (sum_{slots of d} dinv[s]*x[s]) @ (W_gcn@W_act) + (b_gcn@W_act + b_act)
  - 8 cores each own 6250 destination nodes (node -> core node%8, strided so
    each block's self-loops split evenly across src halves; 49 blocks of 128).
  - Edges (incl. self loops) are bucketed by (core, dst-block, src-half) on
    the host (integer metadata only) into a static per-group tile schedule
    (ntile/mcq = cross-core max, data-driven).
  - On device: dma_gather fetches fp16 x rows (256B) per slot from one of
    two half-tables (int16 index limit, <=1024 idx per call = SWDGE ring cap,
    4 SWDGE queues for parallel descriptor generation across Q7 cpu pairs).
    Trailing pad slots use idx=-1 so the ucode skips their descriptors
    entirely (the SWDGE per-descriptor cost is the kernel's critical path);
    their stale SBUF rows are masked by sel=0 (dstloc=-1).
  - A fp16 one-hot selection matrix, pre-scaled by dinv[src] (folded in via
    a second DVE pass), routes each slot to its dst row via TensorE matmuls
    accumulated in fp32 PSUM; per block: scale by dinv[dst], multiply by the
    fused W = W_gcn@W_act (fp32), add fused bias, write out.
"""

import os
import sys

for _p in ("/opt/trn_rl_repo",):
    if _p not in sys.path and os.path.isdir(_p):
        sys.path.insert(0, _p)

import numpy as np

# ---- problem constants (hardcoded per spec) ----
N, E, F, HID, A = 50000, 600000, 128, 128, 64
P = 128                      # partitions
NCORE = 8
DST_PER_CORE = 6250          # N / 8
NBLK = 49                    # ceil(6250/128) dst blocks per core
OUT_ROWS = NBLK * P          # 6272 padded out rows per core
HALF = 25152                 # nodes [0,HALF) in table A, [HALF,...) in table B
XROWS = 25216                # rows per half table (HALF + 64 zero pad rows)
ZROW_A = 25152               # a zero row in table A (explicit pad row)
ZROW_B = 25024               # node 50176 -> xb row 25024 (zero: node >= N)
MAXTPG = 8                   # hard cap: 1024 slots per gather call (ring cap)
XG_BUFS = 3                  # xg buffer rotation depth
CHUNK = 4                    # dst blocks per compute chunk
_CHUNKS = [(c * CHUNK, min(CHUNK, NBLK - c * CHUNK)) for c in range((NBLK + CHUNK - 1) // CHUNK)]

_prog_cache = {}


def _build_program(ntile, mcq):
    """Build the Bass program shared by all 8 cores.

    ntile: tuple of 98 ints — tiles (of 128 slots) per (block, half) group,
    group index g = blk*2 + half.
    mcq: tuple of 98 ints — gather count per group (cross-core max rounded
    up to 32; <= ntile*128). The gather's count register must equal the
    number of descriptors actually generated (SEQ ring accounting), so it
    is a shared constant: each core pads its slots with zero-row idxs up to
    mcq[g] and idx=-1 beyond (skipped)."""
    key = (tuple(ntile), tuple(mcq))
    if key in _prog_cache:
        return _prog_cache[key]

    import concourse.bacc as bacc
    import concourse.mybir as mybir
    import concourse.tile as tile
    from concourse.masks import make_identity

    f32 = mybir.dt.float32
    bf16 = mybir.dt.float16  # message dtype: fp16 (10-bit mantissa, values are O(1))
    i32 = mybir.dt.int32
    i16 = mybir.dt.int16
    Alu = mybir.AluOpType
    Act = mybir.ActivationFunctionType

    col_start = np.concatenate([[0], np.cumsum(ntile)]).astype(int)
    tot_col = int(col_start[-1])
    tot_slot = tot_col * P

    nc = bacc.Bacc(None, num_swdge_queues=4)

    xa = nc.dram_tensor("xa", [XROWS, F], bf16, kind="ExternalInput")
    xb = nc.dram_tensor("xb", [XROWS, F], bf16, kind="ExternalInput")
    idxs = nc.dram_tensor("idxs", [P, tot_slot // 16], i16, kind="ExternalInput")
    dstloc = nc.dram_tensor("dstloc", [P, tot_col], bf16, kind="ExternalInput")
    degsrc = nc.dram_tensor("degsrc", [P, tot_col], f32, kind="ExternalInput")
    degdst = nc.dram_tensor("degdst", [P, NBLK], f32, kind="ExternalInput")
    wgT = nc.dram_tensor("wgT", [HID, F], f32, kind="ExternalInput")
    wact = nc.dram_tensor("wact", [HID, A], f32, kind="ExternalInput")
    bgcn = nc.dram_tensor("bgcn", [HID, 1], f32, kind="ExternalInput")
    bact = nc.dram_tensor("bact", [1, A], f32, kind="ExternalInput")
    out = nc.dram_tensor("out", [OUT_ROWS, A], f32, kind="ExternalOutput")

    with tile.TileContext(nc) as tc:
        with (
            tc.tile_pool(name="const", bufs=1) as cpool,
            tc.tile_pool(name="cpsum", bufs=1, space="PSUM") as cpsum,
            tc.tile_pool(name="sel", bufs=2) as spool,
            tc.tile_pool(name="acc", bufs=2, space="PSUM") as accpool,
            tc.tile_pool(name="accT", bufs=2, space="PSUM") as accTpool,
            tc.tile_pool(name="outp", bufs=2, space="PSUM") as outppool,
            tc.tile_pool(name="flush", bufs=3) as fpool,
        ):
            # ---- constants / prologue ----
            # idx table loaded in per-chunk slices so the first gathers
            # don't wait on the whole 1.5MB transfer
            idx_sb = cpool.tile([P, tot_slot // 16], i16)
            for (b0, ncb) in _CHUNKS:
                a = int(col_start[b0 * 2]) * P // 16
                z = int(col_start[(b0 + ncb) * 2]) * P // 16
                nc.sync.dma_start(out=idx_sb[:, a:z], in_=idxs[:, a:z])

            # persistent gather buffers, rotated across chunks and zeroed once
            # (first in the DVE stream so the first gathers aren't gated behind
            # the rest of the prologue): pad slots are never gathered (idx=-1
            # tails are skipped), so every byte the aggregation matmul can
            # read must start finite
            maxc = max(int(col_start[(b0 + ncb) * 2] - col_start[b0 * 2])
                       for (b0, ncb) in _CHUNKS)
            xg_bufs = []
            for _bi in range(XG_BUFS):
                xg_b = cpool.tile([P, maxc, F], bf16, tag=f"xgb{_bi}")
                nc.vector.memset(xg_b[:], 0.0)
                xg_bufs.append(xg_b)

            dstloc_bf = cpool.tile([P, tot_col], bf16)
            nc.sync.dma_start(out=dstloc_bf[:], in_=dstloc[:])

            dinvsrc = cpool.tile([P, tot_col], f32)
            nc.sync.dma_start(out=dinvsrc[:], in_=degsrc[:])
            nc.vector.reciprocal(out=dinvsrc[:], in_=dinvsrc[:])
            nc.scalar.activation(dinvsrc[:], dinvsrc[:], Act.Sqrt)
            dinv16 = cpool.tile([P, tot_col], bf16)
            nc.vector.tensor_copy(out=dinv16[:], in_=dinvsrc[:])

            dinvdst = cpool.tile([P, NBLK], f32)
            nc.sync.dma_start(out=dinvdst[:], in_=degdst[:])
            nc.vector.reciprocal(out=dinvdst[:], in_=dinvdst[:])
            nc.scalar.activation(dinvdst[:], dinvdst[:], Act.Sqrt)

            ident = cpool.tile([P, P], f32)
            make_identity(nc, ident[:])

            iota_i = cpool.tile([P, P], i32)
            nc.gpsimd.iota(iota_i[:], pattern=[[1, P]], base=0, channel_multiplier=0)
            iota_bf = cpool.tile([P, P], bf16)
            nc.vector.tensor_copy(out=iota_bf[:], in_=iota_i[:])

            wgT_sb = cpool.tile([HID, F], f32)
            nc.sync.dma_start(out=wgT_sb[:], in_=wgT[:])
            wact_sb = cpool.tile([HID, A], f32)
            nc.sync.dma_start(out=wact_sb[:], in_=wact[:])
            wf_ps = cpsum.tile([F, A], f32, space="PSUM", tag="cps")
            nc.tensor.matmul(wf_ps[:], lhsT=wgT_sb[:], rhs=wact_sb[:], start=True, stop=True)
            wf_sb = cpool.tile([F, A], f32)
            nc.vector.tensor_copy(out=wf_sb[:], in_=wf_ps[:])

            bgcn_sb = cpool.tile([HID, 1], f32)
            nc.sync.dma_start(out=bgcn_sb[:], in_=bgcn[:])
            bact_sb = cpool.tile([1, A], f32)
            nc.sync.dma_start(out=bact_sb[:], in_=bact[:])
            cb_ps = cpsum.tile([1, A], f32, space="PSUM", tag="cps")
            nc.tensor.matmul(cb_ps[:], lhsT=bgcn_sb[:], rhs=wact_sb[:], start=True, stop=True)
            cb_sb = cpool.tile([1, A], f32)
            nc.vector.tensor_copy(out=cb_sb[:], in_=cb_ps[:])
            nc.vector.tensor_tensor(out=cb_sb[:], in0=cb_sb[:], in1=bact_sb[:], op=Alu.add)
            ones_row = cpool.tile([1, P], f32)
            nc.vector.memset(ones_row[:], 1.0)
            num_regs = {int(v): nc.gpsimd.to_reg(int(v))
                        for v in sorted(set(int(t) for t in mcq))}

            # ---- main loop over chunks of dst blocks ----
            qn = 0
            for ci, (b0, ncb) in enumerate(_CHUNKS):
                c0 = int(col_start[b0 * 2])
                ncols = int(col_start[(b0 + ncb) * 2] - c0)
                xg = xg_bufs[ci % XG_BUFS]
                for i in range(ncb):
                    for h, tab in ((0, xa), (1, xb)):
                        g = (b0 + i) * 2 + h
                        nt = int(ntile[g])
                        num = nt * P
                        crel = int(col_start[g]) - c0
                        s0 = int(col_start[g]) * P
                        # descriptors generated: mcq[g] (trailing idx=-1 pads
                        # beyond it are trimmed by the ucode; the count reg
                        # must match exactly or SEQ ring accounting desyncs)
                        nc.gpsimd.dma_gather(
                            xg[:, crel: crel + nt, :],
                            tab[:],
                            idx_sb[:, s0 // 16: (s0 + num) // 16],
                            num,
                            num_regs[int(mcq[g])],
                            F,
                            queue_num=qn % 4,
                        )
                        qn += 1
                # fp16 one-hot selection (broadcast TT on DVE), then fold in
                # dinv[src] so the gathered fp16 rows are consumed unscaled
                sel = spool.tile([P, ncols, P], bf16, tag="sel")
                nc.vector.tensor_tensor(
                    out=sel[:],
                    in0=dstloc_bf[:, c0:c0 + ncols].unsqueeze(2).broadcast_to([P, ncols, P]),
                    in1=iota_bf[:].unsqueeze(1).broadcast_to([P, ncols, P]),
                    op=Alu.is_equal,
                )
                nc.vector.tensor_tensor(
                    out=sel[:],
                    in0=sel[:],
                    in1=dinv16[:, c0:c0 + ncols].unsqueeze(2).broadcast_to([P, ncols, P]),
                    op=Alu.mult,
                )
                for i in range(ncb):
                    b = b0 + i
                    gA, gB = b * 2, b * 2 + 1
                    acc = accpool.tile([P, P], f32, space="PSUM", tag="acc")
                    cols = list(range(int(col_start[gA]) - c0, int(col_start[gB + 1]) - c0))
                    for j, col in enumerate(cols):
                        nc.tensor.matmul(
                            acc[:],
                            lhsT=sel[:, col, :],
                            rhs=xg[:, col, :],
                            start=(j == 0),
                            stop=(j == len(cols) - 1),
                        )
                    # flush block b
                    accS = fpool.tile([P, P], f32, tag="accS")
                    nc.scalar.activation(accS[:], acc[:], Act.Copy, scale=dinvdst[:, b:b + 1])
                    accTp = accTpool.tile([P, P], f32, space="PSUM", tag="accT")
                    nc.tensor.transpose(accTp[:], accS[:], ident[:])
                    accT = fpool.tile([P, P], f32, tag="accTs")
                    nc.scalar.activation(accT[:], accTp[:], Act.Copy)
                    outp = outppool.tile([P, A], f32, space="PSUM", tag="outp")
                    nc.tensor.matmul(outp[:], lhsT=accT[:], rhs=wf_sb[:], start=True, stop=False)
                    nc.tensor.matmul(outp[:], lhsT=ones_row[:], rhs=cb_sb[:], start=False, stop=True)
                    out_sb = fpool.tile([P, A], f32, tag="outs")
                    nc.scalar.activation(out_sb[:], outp[:], Act.Copy)
                    nc.sync.dma_start(out=out[b * P:(b + 1) * P, :], in_=out_sb[:])

    nc.compile()
    _prog_cache[key] = nc
    return nc


def _preprocess(x, edge_index):
    """Host-side sharding: bucket edges by (core, dst block, src half) and
    build the static padded slot arrays. Integer/layout work only."""
    src = np.asarray(edge_index[0], dtype=np.int64)
    dst = np.asarray(edge_index[1], dtype=np.int64)

    in_deg = np.bincount(dst, minlength=N).astype(np.int64)
    deg_tot = in_deg + 1  # self loop

    # all slots: real edges then self loops
    s_src = np.concatenate([src, np.arange(N, dtype=np.int64)])
    s_dst = np.concatenate([dst, np.arange(N, dtype=np.int64)])

    # strided dst sharding (node -> core node%8): spreads each block's 128
    # self-loops evenly over the two src halves, tightening the cross-core
    # max group counts that size the gather schedule
    core = s_dst % NCORE
    loc = s_dst // NCORE
    blk = loc >> 7
    dloc = loc & 127
    half = (s_src >= HALF).astype(np.int64)
    rowid = s_src - HALF * half
    dsrc = deg_tot[s_src]

    # group = (core, blk, half); position within group via stable sort
    g = (core * NBLK + blk) * 2 + half
    order = np.argsort(g, kind="stable")
    g_sorted = g[order]
    cnt = np.bincount(g_sorted, minlength=NCORE * NBLK * 2)
    # static tile schedule: cross-core max per (blk, half) group
    cnt2 = cnt.reshape(NCORE, NBLK * 2)
    ntile = np.maximum(1, -(-cnt2.max(axis=0) // P))  # [98]
    if ntile.max() > MAXTPG:
        raise RuntimeError(f"group needs {ntile.max()} tiles > {MAXTPG}")
    col_start = np.concatenate([[0], np.cumsum(ntile)]).astype(np.int64)
    tot_col = int(col_start[-1])
    tot_slot = tot_col * P

    starts = np.zeros_like(cnt)
    starts[1:] = np.cumsum(cnt)[:-1]
    pos_in_group = np.arange(len(order)) - starts[g_sorted]

    blk_s = blk[order]
    half_s = half[order]
    g2 = blk_s * 2 + half_s
    col = col_start[g2] + (pos_in_group >> 7)
    p = pos_in_group & 127
    flat = col * P + p  # slot id within core

    core_s = core[order]
    rowid_s = rowid[order]
    dloc_s = dloc[order]
    dsrc_s = dsrc[order]

    # Pad structure per (core, group): [real slots | ZROW pads up to mcq[g]
    # | idx=-1]. mcq = cross-core max count rounded up to 32 (few distinct
    # register constants). The gather generates exactly mcq[g] descriptors
    # on every core (trailing -1s are trimmed by the ucode); sel=0
    # (dstloc=-1) masks all pad rows.
    mcq = np.minimum(-(-cnt2.max(axis=0) // 32) * 32, ntile * P).astype(np.int64)
    colg = np.repeat(np.arange(NBLK * 2), ntile)          # group of each column
    slotg = np.repeat(colg, P)                            # group of each slot
    g_off = np.concatenate([[0], np.cumsum(ntile * P)])   # slot base per group
    in_mcq = (np.arange(tot_slot) - g_off[slotg]) < mcq[slotg]
    zrow = np.where(slotg % 2 == 1, ZROW_B, ZROW_A).astype(np.int16)
    idx_arr = np.empty((NCORE, tot_slot), dtype=np.int16)
    idx_arr[:] = np.where(in_mcq, zrow, np.int16(-1))[None, :]
    dst_arr = np.full((NCORE, tot_slot), -1.0, dtype=np.float16)
    dsc_arr = np.ones((NCORE, tot_slot), dtype=np.float32)

    lin = core_s * tot_slot + flat
    idx_arr.reshape(-1)[lin] = rowid_s.astype(np.int16)
    dst_arr.reshape(-1)[lin] = dloc_s.astype(np.float16)
    dsc_arr.reshape(-1)[lin] = dsrc_s.astype(np.float32)

    # idxs: 16-partition wrap replicated 8x -> [128, tot_slot//16]
    idx_wrap = idx_arr.reshape(NCORE, tot_slot // 16, 16).transpose(0, 2, 1)
    idx_rep = np.tile(idx_wrap, (1, 8, 1)).copy()

    # dstloc/degsrc: [128, tot_col] with value at [p, col]
    dst_pc = dst_arr.reshape(NCORE, tot_col, P).transpose(0, 2, 1).copy()
    dsc_pc = dsc_arr.reshape(NCORE, tot_col, P).transpose(0, 2, 1).copy()

    # degdst: [NCORE, 128, NBLK]
    degdst = np.ones((NCORE, P, NBLK), dtype=np.float32)
    node = np.arange(N, dtype=np.int64)
    nc_ = node % NCORE
    nl = node // NCORE
    degdst[nc_, nl & 127, nl >> 7] = deg_tot.astype(np.float32)

    # x half tables (fp16, zero padded): 256B gather rows
    x16 = np.asarray(x, dtype=np.float16)
    xa = np.zeros((XROWS, F), dtype=np.float16)
    xa[:HALF] = x16[:HALF]
    xb = np.zeros((XROWS, F), dtype=np.float16)
    xb[: N - HALF] = x16[HALF:]

    return ntile, mcq, xa, xb, idx_rep, dst_pc, dsc_pc, degdst


def kernel(x, edge_index, W_gcn, b_gcn, W_act, b_act):
    from concourse.bass_utils import run_bass_kernel_spmd

    x = np.ascontiguousarray(np.asarray(x, dtype=np.float32))
    ntile, mcq, xa, xb, idx_rep, dst_pc, dsc_pc, degdst = _preprocess(x, edge_index)

    wgT = np.ascontiguousarray(np.asarray(W_gcn, dtype=np.float32).T)
    wact = np.ascontiguousarray(np.asarray(W_act, dtype=np.float32))
    bg = np.ascontiguousarray(np.asarray(b_gcn, dtype=np.float32).reshape(HID, 1))
    ba = np.ascontiguousarray(np.asarray(b_act, dtype=np.float32).reshape(1, A))

    nc = _build_program(tuple(int(v) for v in ntile), tuple(int(v) for v in mcq))
    in_maps = [
        {
            "xa": xa,
            "xb": xb,
            "idxs": idx_rep[c],
            "dstloc": dst_pc[c],
            "degsrc": dsc_pc[c],
            "degdst": degdst[c],
            "wgT": wgT,
            "wact": wact,
            "bgcn": bg,
            "bact": ba,
        }
        for c in range(NCORE)
    ]
    trace = bool(os.environ.get("GCN_TRACE"))
    res = run_bass_kernel_spmd(nc, in_maps, core_ids=list(range(NCORE)), trace=trace)
    kernel.last_results = res

    out = np.empty((N, A), dtype=np.float32)
    for c in range(NCORE):
        out[c::NCORE] = res.results[c]["out"][:DST_PER_CORE]
    return out



# revision 6
# speedup vs baseline: 1.0964x; 1.0964x over previous
"""GCN dialogue manager kernel for 8 trn2 NeuronCores.

Math (reference):
    h   = x @ W_gcn
    deg = in_deg(dst) + 1   (self loops)
    dinv = rsqrt(deg)
    agg[d] = sum_{e:(s->d)} dinv[s]*dinv[d]*h[s] + dinv[d]^2*h[d] + b_gcn
    out = agg @ W_act + b_act

Kernel strategy (dst-sharded, edges partitioned by destination):
    out[d] = (sum_{slots of d} w_slot*x[s]) @ (W_gcn@W_act) + (b_gcn@W_act + b_act)
    with w_slot = rsqrt(deg[s]*deg[d]) folded per slot (self loop = dinv[d]^2).
  - 8 cores each own 6250 destination nodes (node -> core node%8, strided so
    each block's self-loops split evenly across src halves, 49 blocks of 128).
  - Edges (incl. self loops) are bucketed by (core, dst-block, src-half) on
    the host (integer metadata only) into a static per-group tile schedule
    (ntile/mcq = cross-core max, data-driven).
  - On device: dma_gather fetches fp16 x rows (256B) per slot from one of
    two half-tables (int16 index limit, <=1024 idx per call = SWDGE ring cap,
    4 SWDGE queues). Trailing pad slots use idx=-1 (no descriptors); their
    stale SBUF rows are masked by sel=0.
  - sel (the one-hot slot->dst routing matrix, value w_slot) is built per
    chunk WITHOUT the slow broadcast tensor_tensor passes:
      * even chunks: gpsimd local_scatter (windows of <=14 even columns,
        per-partition idx = col_in_window*128 + dstloc, zero-fill included)
      * odd chunks:  one DVE tensor_scalar per column:
        sel[:,c,:] = (iota == dstloc[:,c]) * w[:,c] (per-partition scalars)
  - Aggregation per block: matmul(acc, lhsT=xg_col, rhs=sel_col) accumulated
    in PSUM -> acc[F, dst] (feature-major: no transpose needed downstream).
  - Flush per block: ACT copy acc PSUM->SBUF, matmul with fused
    W = W_gcn@W_act (fp32) plus fused-bias matmul, write out.
"""

import os
import sys

for _p in ("/opt/trn_rl_repo",):
    if _p not in sys.path and os.path.isdir(_p):
        sys.path.insert(0, _p)

import numpy as np

# ---- problem constants (hardcoded per spec) ----
N, E, F, HID, A = 50000, 600000, 128, 128, 64
P = 128                      # partitions
NCORE = 8
DST_PER_CORE = 6250          # N / 8
NBLK = 49                    # ceil(6250/128) dst blocks per core
OUT_ROWS = NBLK * P          # 6272 padded out rows per core
HALF = 25152                 # nodes [0,HALF) in table A, [HALF,...) in table B
XROWS = 25216                # rows per half table (HALF + 64 zero pad rows)
ZROW_A = 25152               # a zero row in table A (explicit pad row)
ZROW_B = 25024               # node 50176 -> xb row 25024 (zero: node >= N)
MAXTPG = 8                   # hard cap: 1024 slots per gather call (ring cap)
XG_BUFS = 3                  # xg buffer rotation depth
CHUNK = 4                    # dst blocks per compute chunk
WMAX = 14                    # local_scatter window: <=14 cols (num_elems<2048)
SCAT_MOD = 2                 # chunk ci uses local_scatter iff ci % SCAT_MOD == 0
_CHUNKS = [(c * CHUNK, min(CHUNK, NBLK - c * CHUNK)) for c in range((NBLK + CHUNK - 1) // CHUNK)]

_prog_cache = {}


def _windows(ncols):
    """Even-sized local_scatter windows over a chunk's columns. ncols is
    always even (ntile parity fix in _preprocess)."""
    assert ncols % 2 == 0, ncols
    out = []
    w0 = 0
    while w0 < ncols:
        nw = min(WMAX, ncols - w0)
        out.append((w0, nw))
        w0 += nw
    return out


def _build_program(ntile, mcq):
    """Build the Bass program shared by all 8 cores.

    ntile: tuple of 98 ints — tiles (of 128 slots) per (block, half) group,
    group index g = blk*2 + half.
    mcq: tuple of 98 ints — gather count per group (cross-core max rounded
    up to 32; <= ntile*128). The gather's count register must equal the
    number of descriptors actually generated, so it is a shared constant:
    each core pads its slots with zero-row idxs up to mcq[g] and idx=-1
    beyond (skipped)."""
    key = (tuple(ntile), tuple(mcq))
    if key in _prog_cache:
        return _prog_cache[key]

    import concourse.bacc as bacc
    import concourse.mybir as mybir
    import concourse.tile as tile

    f32 = mybir.dt.float32
    bf16 = mybir.dt.float16  # message dtype: fp16 (10-bit mantissa, values are O(1))
    i32 = mybir.dt.int32
    i16 = mybir.dt.int16
    Alu = mybir.AluOpType
    Act = mybir.ActivationFunctionType

    col_start = np.concatenate([[0], np.cumsum(ntile)]).astype(int)
    tot_col = int(col_start[-1])
    tot_slot = tot_col * P

    nc = bacc.Bacc(None, num_swdge_queues=4)

    xa = nc.dram_tensor("xa", [XROWS, F], bf16, kind="ExternalInput")
    xb = nc.dram_tensor("xb", [XROWS, F], bf16, kind="ExternalInput")
    idxs = nc.dram_tensor("idxs", [P, tot_slot // 16], i16, kind="ExternalInput")
    dstloc = nc.dram_tensor("dstloc", [P, tot_col], f32, kind="ExternalInput")
    scatidx = nc.dram_tensor("scatidx", [P, tot_col], i16, kind="ExternalInput")
    degprod = nc.dram_tensor("degprod", [P, tot_col], f32, kind="ExternalInput")
    wgT = nc.dram_tensor("wgT", [HID, F], f32, kind="ExternalInput")
    wact = nc.dram_tensor("wact", [HID, A], f32, kind="ExternalInput")
    bgcn = nc.dram_tensor("bgcn", [HID, 1], f32, kind="ExternalInput")
    bact = nc.dram_tensor("bact", [1, A], f32, kind="ExternalInput")
    out = nc.dram_tensor("out", [OUT_ROWS, A], f32, kind="ExternalOutput")

    with tile.TileContext(nc) as tc:
        with (
            tc.tile_pool(name="const", bufs=1) as cpool,
            tc.tile_pool(name="cpsum", bufs=1, space="PSUM") as cpsum,
            tc.tile_pool(name="sel", bufs=2) as spool,
            tc.tile_pool(name="acc", bufs=2, space="PSUM") as accpool,
            tc.tile_pool(name="outp", bufs=2, space="PSUM") as outppool,
            tc.tile_pool(name="flush", bufs=3) as fpool,
        ):
            # ---- constants / prologue ----
            # idx table loaded in per-chunk slices so the first gathers
            # don't wait on the whole transfer
            idx_sb = cpool.tile([P, tot_slot // 16], i16)
            for (b0, ncb) in _CHUNKS:
                a = int(col_start[b0 * 2]) * P // 16
                z = int(col_start[(b0 + ncb) * 2]) * P // 16
                nc.sync.dma_start(out=idx_sb[:, a:z], in_=idxs[:, a:z])

            # persistent gather buffers, rotated across chunks and zeroed once
            # (pad slots are never gathered, so every byte the aggregation
            # matmul can read must start finite; sel=0 masks their values)
            maxc = max(int(col_start[(b0 + ncb) * 2] - col_start[b0 * 2])
                       for (b0, ncb) in _CHUNKS)
            xg_bufs = []
            for _bi in range(XG_BUFS):
                xg_b = cpool.tile([P, maxc, F], bf16, tag=f"xgb{_bi}")
                nc.vector.memset(xg_b[:], 0.0)
                xg_bufs.append(xg_b)

            # per-slot routing metadata
            dstloc_sb = cpool.tile([P, tot_col], f32)
            nc.sync.dma_start(out=dstloc_sb[:], in_=dstloc[:])
            scat_sb = cpool.tile([P, tot_col], i16)
            nc.sync.dma_start(out=scat_sb[:], in_=scatidx[:])

            # per-slot weight w = rsqrt(deg_src*deg_dst)
            w32 = cpool.tile([P, tot_col], f32)
            nc.sync.dma_start(out=w32[:], in_=degprod[:])
            nc.vector.reciprocal(out=w32[:], in_=w32[:])
            nc.scalar.activation(w32[:], w32[:], Act.Sqrt)
            w16 = cpool.tile([P, tot_col], bf16)
            nc.vector.tensor_copy(out=w16[:], in_=w32[:])

            iota_i = cpool.tile([P, P], i32)
            nc.gpsimd.iota(iota_i[:], pattern=[[1, P]], base=0, channel_multiplier=0)
            iota_bf = cpool.tile([P, P], bf16)
            nc.vector.tensor_copy(out=iota_bf[:], in_=iota_i[:])

            wgT_sb = cpool.tile([HID, F], f32)
            nc.sync.dma_start(out=wgT_sb[:], in_=wgT[:])
            wact_sb = cpool.tile([HID, A], f32)
            nc.sync.dma_start(out=wact_sb[:], in_=wact[:])
            wf_ps = cpsum.tile([F, A], f32, space="PSUM", tag="cps")
            nc.tensor.matmul(wf_ps[:], lhsT=wgT_sb[:], rhs=wact_sb[:], start=True, stop=True)
            wf_sb = cpool.tile([F, A], f32)
            nc.vector.tensor_copy(out=wf_sb[:], in_=wf_ps[:])

            bgcn_sb = cpool.tile([HID, 1], f32)
            nc.sync.dma_start(out=bgcn_sb[:], in_=bgcn[:])
            bact_sb = cpool.tile([1, A], f32)
            nc.sync.dma_start(out=bact_sb[:], in_=bact[:])
            cb_ps = cpsum.tile([1, A], f32, space="PSUM", tag="cps")
            nc.tensor.matmul(cb_ps[:], lhsT=bgcn_sb[:], rhs=wact_sb[:], start=True, stop=True)
            cb_sb = cpool.tile([1, A], f32)
            nc.vector.tensor_copy(out=cb_sb[:], in_=cb_ps[:])
            nc.vector.tensor_tensor(out=cb_sb[:], in0=cb_sb[:], in1=bact_sb[:], op=Alu.add)
            ones_row = cpool.tile([1, P], f32)
            nc.vector.memset(ones_row[:], 1.0)
            num_regs = {int(v): nc.gpsimd.to_reg(int(v))
                        for v in sorted(set(int(t) for t in mcq))}

            # ---- main loop over chunks of dst blocks ----
            qn = 0
            for ci, (b0, ncb) in enumerate(_CHUNKS):
                c0 = int(col_start[b0 * 2])
                ncols = int(col_start[(b0 + ncb) * 2] - c0)
                xg = xg_bufs[ci % XG_BUFS]
                for i in range(ncb):
                    for h, tab in ((0, xa), (1, xb)):
                        g = (b0 + i) * 2 + h
                        nt = int(ntile[g])
                        num = nt * P
                        crel = int(col_start[g]) - c0
                        s0 = int(col_start[g]) * P
                        nc.gpsimd.dma_gather(
                            xg[:, crel: crel + nt, :],
                            tab[:],
                            idx_sb[:, s0 // 16: (s0 + num) // 16],
                            num,
                            num_regs[int(mcq[g])],
                            F,
                            queue_num=qn % 4,
                        )
                        qn += 1
                # sel: one-hot routing matrix with w folded in
                sel = spool.tile([P, maxc, P], bf16, tag="sel")
                if ci % SCAT_MOD == 0:
                    for (w0, nw) in _windows(ncols):
                        nc.gpsimd.local_scatter(
                            sel[:, w0:w0 + nw, :],
                            w16[:, c0 + w0: c0 + w0 + nw],
                            scat_sb[:, c0 + w0: c0 + w0 + nw],
                            channels=P,
                            num_elems=nw * P,
                            num_idxs=nw,
                        )
                else:
                    for c in range(ncols):
                        nc.vector.tensor_scalar(
                            out=sel[:, c, :],
                            in0=iota_bf[:],
                            scalar1=dstloc_sb[:, c0 + c: c0 + c + 1],
                            scalar2=w32[:, c0 + c: c0 + c + 1],
                            op0=Alu.is_equal,
                            op1=Alu.mult,
                        )
                for i in range(ncb):
                    b = b0 + i
                    gA, gB = b * 2, b * 2 + 1
                    # acc[F, dst] = sum_cols xg_col^T @ sel_col  (PSUM accum)
                    acc = accpool.tile([P, P], f32, space="PSUM", tag="acc")
                    cols = list(range(int(col_start[gA]) - c0, int(col_start[gB + 1]) - c0))
                    for j, col in enumerate(cols):
                        nc.tensor.matmul(
                            acc[:],
                            lhsT=xg[:, col, :],
                            rhs=sel[:, col, :],
                            start=(j == 0),
                            stop=(j == len(cols) - 1),
                        )
                    # flush block b: acc is feature-major, no transpose needed
                    accS = fpool.tile([P, P], f32, tag="accS")
                    nc.scalar.activation(accS[:], acc[:], Act.Copy)
                    outp = outppool.tile([P, A], f32, space="PSUM", tag="outp")
                    nc.tensor.matmul(outp[:], lhsT=accS[:], rhs=wf_sb[:], start=True, stop=False)
                    nc.tensor.matmul(outp[:], lhsT=ones_row[:], rhs=cb_sb[:], start=False, stop=True)
                    out_sb = fpool.tile([P, A], f32, tag="outs")
                    nc.scalar.activation(out_sb[:], outp[:], Act.Copy)
                    nc.sync.dma_start(out=out[b * P:(b + 1) * P, :], in_=out_sb[:])

    nc.compile()
    _prog_cache[key] = nc
    return nc


def _preprocess(x, edge_index):
    """Host-side sharding: bucket edges by (core, dst block, src half) and
    build the static padded slot arrays. Integer/layout work only."""
    src = np.asarray(edge_index[0], dtype=np.int64)
    dst = np.asarray(edge_index[1], dtype=np.int64)

    in_deg = np.bincount(dst, minlength=N).astype(np.int64)
    deg_tot = in_deg + 1  # self loop

    # all slots: real edges then self loops
    s_src = np.concatenate([src, np.arange(N, dtype=np.int64)])
    s_dst = np.concatenate([dst, np.arange(N, dtype=np.int64)])

    # strided dst sharding (node -> core node%8): spreads each block's 128
    # self-loops evenly over the two src halves, tightening the cross-core
    # max group counts that size the gather schedule
    core = s_dst % NCORE
    loc = s_dst // NCORE
    blk = loc >> 7
    dloc = loc & 127
    half = (s_src >= HALF).astype(np.int64)
    rowid = s_src - HALF * half
    dprod = deg_tot[s_src] * deg_tot[s_dst]

    # group = (core, blk, half); position within group via stable sort
    g = (core * NBLK + blk) * 2 + half
    order = np.argsort(g, kind="stable")
    g_sorted = g[order]
    cnt = np.bincount(g_sorted, minlength=NCORE * NBLK * 2)
    # static tile schedule: cross-core max per (blk, half) group
    cnt2 = cnt.reshape(NCORE, NBLK * 2)
    ntile = np.maximum(1, -(-cnt2.max(axis=0) // P))  # [98]
    if ntile.max() > MAXTPG:
        raise RuntimeError(f"group needs {ntile.max()} tiles > {MAXTPG}")
    # force each chunk's column count even (local_scatter windows need even
    # sizes): bump the smallest group in the chunk by one tile of trailing
    # idx=-1 slots (no gather descriptors, one extra masked matmul column)
    for (b0, ncb) in _CHUNKS:
        gs = slice(b0 * 2, (b0 + ncb) * 2)
        if int(ntile[gs].sum()) % 2 == 1:
            gi = b0 * 2 + int(np.argmin(ntile[gs]))
            assert ntile[gi] < MAXTPG
            ntile[gi] += 1
    col_start = np.concatenate([[0], np.cumsum(ntile)]).astype(np.int64)
    tot_col = int(col_start[-1])
    tot_slot = tot_col * P

    starts = np.zeros_like(cnt)
    starts[1:] = np.cumsum(cnt)[:-1]
    pos_in_group = np.arange(len(order)) - starts[g_sorted]

    blk_s = blk[order]
    half_s = half[order]
    g2 = blk_s * 2 + half_s
    col = col_start[g2] + (pos_in_group >> 7)
    p = pos_in_group & 127
    flat = col * P + p  # slot id within core

    core_s = core[order]
    rowid_s = rowid[order]
    dloc_s = dloc[order]
    dprod_s = dprod[order]

    # Pad structure per (core, group): [real slots | ZROW pads up to mcq[g]
    # | idx=-1]. mcq = cross-core max count rounded up to 32 (few distinct
    # register constants). The gather generates exactly mcq[g] descriptors
    # on every core (trailing -1s are trimmed by the ucode); sel=0 masks
    # all pad rows.
    mcq = np.minimum(-(-cnt2.max(axis=0) // 32) * 32, ntile * P).astype(np.int64)
    colg = np.repeat(np.arange(NBLK * 2), ntile)          # group of each column
    slotg = np.repeat(colg, P)                            # group of each slot
    g_off = np.concatenate([[0], np.cumsum(ntile * P)])   # slot base per group
    in_mcq = (np.arange(tot_slot) - g_off[slotg]) < mcq[slotg]
    zrow = np.where(slotg % 2 == 1, ZROW_B, ZROW_A).astype(np.int16)
    idx_arr = np.empty((NCORE, tot_slot), dtype=np.int16)
    idx_arr[:] = np.where(in_mcq, zrow, np.int16(-1))[None, :]
    dst_arr = np.full((NCORE, tot_slot), -1.0, dtype=np.float32)
    dpr_arr = np.ones((NCORE, tot_slot), dtype=np.float32)

    lin = core_s * tot_slot + flat
    idx_arr.reshape(-1)[lin] = rowid_s.astype(np.int16)
    dst_arr.reshape(-1)[lin] = dloc_s.astype(np.float32)
    dpr_arr.reshape(-1)[lin] = dprod_s.astype(np.float32)

    # scatidx: per-slot local_scatter index = win_rel_col*128 + dloc (or -1)
    # windows are a deterministic function of ntile shared with the builder
    colof = np.arange(tot_col)
    win_rel = np.zeros(tot_col, dtype=np.int64)
    for (b0, ncb) in _CHUNKS:
        c0 = int(col_start[b0 * 2])
        ncols = int(col_start[(b0 + ncb) * 2] - c0)
        for (w0, nw) in _windows(ncols):
            lo, hi = c0 + w0, c0 + w0 + nw
            win_rel[lo:hi] = colof[lo:hi] - (c0 + w0)
    scat_arr = np.where(
        dst_arr.reshape(NCORE, tot_col, P) >= 0,
        (win_rel[None, :, None] * P + dst_arr.reshape(NCORE, tot_col, P)).astype(np.int16),
        np.int16(-1),
    ).astype(np.int16)

    # idxs: 16-partition wrap replicated 8x -> [128, tot_slot//16]
    idx_wrap = idx_arr.reshape(NCORE, tot_slot // 16, 16).transpose(0, 2, 1)
    idx_rep = np.tile(idx_wrap, (1, 8, 1)).copy()

    # dstloc/degprod/scatidx: [128, tot_col] with value at [p, col]
    dst_pc = dst_arr.reshape(NCORE, tot_col, P).transpose(0, 2, 1).copy()
    dpr_pc = dpr_arr.reshape(NCORE, tot_col, P).transpose(0, 2, 1).copy()
    sct_pc = scat_arr.transpose(0, 2, 1).copy()

    # x half tables (fp16, zero padded): 256B gather rows
    x16 = np.asarray(x, dtype=np.float16)
    xa = np.zeros((XROWS, F), dtype=np.float16)
    xa[:HALF] = x16[:HALF]
    xb = np.zeros((XROWS, F), dtype=np.float16)
    xb[: N - HALF] = x16[HALF:]

    return ntile, mcq, xa, xb, idx_rep, dst_pc, dpr_pc, sct_pc


def kernel(x, edge_index, W_gcn, b_gcn, W_act, b_act):
    from concourse.bass_utils import run_bass_kernel_spmd

    x = np.ascontiguousarray(np.asarray(x, dtype=np.float32))
    ntile, mcq, xa, xb, idx_rep, dst_pc, dpr_pc, sct_pc = _preprocess(x, edge_index)

    wgT = np.ascontiguousarray(np.asarray(W_gcn, dtype=np.float32).T)
    wact = np.ascontiguousarray(np.asarray(W_act, dtype=np.float32))
    bg = np.ascontiguousarray(np.asarray(b_gcn, dtype=np.float32).reshape(HID, 1))
    ba = np.ascontiguousarray(np.asarray(b_act, dtype=np.float32).reshape(1, A))

    nc = _build_program(tuple(int(v) for v in ntile), tuple(int(v) for v in mcq))
    in_maps = [
        {
            "xa": xa,
            "xb": xb,
            "idxs": idx_rep[c],
            "dstloc": dst_pc[c],
            "scatidx": sct_pc[c],
            "degprod": dpr_pc[c],
            "wgT": wgT,
            "wact": wact,
            "bgcn": bg,
            "bact": ba,
        }
        for c in range(NCORE)
    ]
    trace = bool(os.environ.get("GCN_TRACE"))
    res = run_bass_kernel_spmd(nc, in_maps, core_ids=list(range(NCORE)), trace=trace)
    kernel.last_results = res

    out = np.empty((N, A), dtype=np.float32)
    for c in range(NCORE):
        out[c::NCORE] = res.results[c]["out"][:DST_PER_CORE]
    return out
